# revision 21
# baseline (speedup 1.0000x reference)
"""VQ codebook assignment (ApplyKmeans) on 8 Trainium2 NeuronCores.

tokens[n] = argmin_k ||x_n - c_k||^2
          = argmin_k (Cnorm_k - 2 x_n.c_k)        (||x_n||^2 constant per row)
          = argmax_k (x_n.c_k - Cnorm_k/2)

Data-parallel: x sharded along N across 8 cores, C/Cnorm replicated.

Per core (16384 rows):
  - host pre-tiles x^T so each [128d, 128n] stationary tile is contiguous
    (fp16; halves HBM traffic, PSUM accumulates fp32)
  - 128 row-tiles; per tile: 1 bias matmul (ones x [-Cnorm/2 hi/lo]) + 8
    accumulating matmuls (x^T chunk stationary, C chunk moving) -> PSUM
    [128, 300] holds val = x.C - Cnorm/2
  - VectorE max8 + max_index -> first-occurrence argmax index per row
  - tokens assembled in SBUF, one DMA out

Row interleaving: row-tile t holds rows {p*128 + t}, so the token buffer
[p, t] DMAs out contiguously in original row order.
"""

import os
import sys

import numpy as np

if "/opt/trn_rl_repo" not in sys.path:
    sys.path.insert(0, "/opt/trn_rl_repo")

import concourse.bass as bass
import concourse.mybir as mybir
import concourse.tile_sem_assignment as _tsa
from concourse.bass_utils import run_bass_kernel_spmd
from concourse.tile import TileContext

_tsa.NUM_HWDGE_SEMS = int(os.environ.get("KM_HW_LANES", "8"))

# Give each HWDGE ring (SP-issued vs ACT-issued DMAs) a disjoint pool of
# completion lanes. Tile's global round-robin otherwise interleaves the
# two rings onto shared lanes, and the lane-order WAW waits then falsely
# serialize one ring behind the other.
_orig_assign_tick = _tsa.TileClockTick._assign_tick


def _assign_tick_lanepools(self, inst):
    try:
        if isinstance(inst, _tsa.DMAInst) and inst.engine != mybir.EngineType.Pool:
            if not hasattr(self, "_lane_ctr"):
                self._lane_ctr = {}
            eng = inst.engine
            n = _tsa.NUM_HWDGE_SEMS
            half = max(1, n // 2)
            pool = (
                list(range(0, half))
                if eng == mybir.EngineType.Activation
                else list(range(half, n))
            )
            c = self._lane_ctr.get(eng, 0)
            self.next_hw_dma_idx = pool[c % len(pool)]
            self._lane_ctr[eng] = c + 1
    except Exception:
        pass
    return _orig_assign_tick(self, inst)


_tsa.TileClockTick._assign_tick = _assign_tick_lanepools

P = 128
D = 1024
K = 300
NCORES = 8
ROWS = 16384            # rows per core
TILES = ROWS // P       # 128 row-tiles per core
GROUPS = 16             # DMA groups per core (1 group = 2 MB fp16)
TPG = TILES // GROUPS   # 8 row-tiles per group
DCH = D // P            # 8 contraction chunks

F16 = mybir.dt.float16
F32 = mybir.dt.float32
I32 = mybir.dt.int32
U32 = mybir.dt.uint32

# Set by kernel() so test.py can read profiling info.
LAST_RESULT = None


def _ensure_ntff_hook():
    """Install antenv.axon_hooks shim so trace=True works under axon."""
    try:
        from antenv.axon_hooks import get_axon_ntff_profile_hook  # noqa: F401

        return
    except ImportError:
        pass
    import types

    import antenv

    try:
        from trn_agent_boot.trn_boot import _ntff_profile_via_ctypes
    except ImportError:
        return
    mod = types.ModuleType("antenv.axon_hooks")
    _hook = [None]
    mod.set_axon_ntff_profile_hook = lambda h: _hook.__setitem__(0, h)
    mod.get_axon_ntff_profile_hook = lambda: _hook[0]
    sys.modules["antenv.axon_hooks"] = mod
    antenv.axon_hooks = mod
    so = "/opt/axon/libaxon_pjrt.so"
    if os.path.exists(so):
        mod.set_axon_ntff_profile_hook(_ntff_profile_via_ctypes(so))


def build_nc(use_act_copy: bool = False) -> bass.Bass:
    nc = bass.Bass()

    xg = nc.declare_dram_parameter("xg", [GROUPS, P, DCH * TPG * P], F16, isOutput=False)
    # consts: [C chunks j=0..7 | bias (-Cnorm/2 hi/lo in rows 0-1) | ones]
    cons = nc.declare_dram_parameter("cons", [P, DCH * K + K + P], F16, isOutput=False)
    out = nc.declare_dram_parameter("out", [P, TILES], I32, isOutput=True)

    OSL = 16  # token output slice, in tiles

    with TileContext(nc) as tc:
        with (
            tc.tile_pool(name="const", bufs=1) as constp,
            tc.tile_pool(name="xp0", bufs=DCH) as xp0,
            tc.tile_pool(name="xp", bufs=3) as xp,
            tc.tile_pool(name="mx", bufs=8) as mxp,
            tc.tile_pool(name="val", bufs=4) as valp,
            tc.tile_pool(name="psum", bufs=8, space="PSUM") as psp,
            tc.tile_pool(name="outp", bufs=1) as outp,
        ):
            # constants in one DMA on the scalar HWDGE ring; x chunks on
            # the sync ring — the two rings issue in parallel.
            cons_t = constp.tile([P, DCH * K + K + P], F16)
            nc.scalar.dma_start(out=cons_t[:], in_=cons[:])
            ctiles = [cons_t[:, j * K : (j + 1) * K] for j in range(DCH)]
            btile = cons_t[:, DCH * K : DCH * K + K]
            otile = cons_t[:, DCH * K + K :]

            # group 0 arrives chunk-by-chunk so the PE can start early
            xch0 = []
            for j in range(DCH):
                cbuf = xp0.tile([P, TPG, P], F16, name="xchunk")
                nc.sync.dma_start(
                    out=cbuf[:],
                    in_=xg[0, :, j * TPG * P : (j + 1) * TPG * P].rearrange(
                        "p (t q) -> p t q", t=TPG
                    ),
                )
                xch0.append(cbuf)

            idxbuf = outp.tile([P, TILES, 8], U32)
            tokbuf = outp.tile([P, TILES], I32)

            for g in range(GROUPS):
                if g == 0:
                    chunk = lambda j, tl: xch0[j][:, tl, :]
                else:
                    xbuf = xp.tile([P, DCH, TPG, P], F16, name="xgrp")
                    eng = nc.sync if g % 2 else nc.scalar
                    eng.dma_start(
                        out=xbuf[:],
                        in_=xg[g].rearrange("p (j t q) -> p j t q", j=DCH, t=TPG),
                    )
                    chunk = lambda j, tl, xbuf=xbuf: xbuf[:, j, tl, :]
                for tl in range(TPG):
                    t = g * TPG + tl
                    psum = psp.tile([P, K], F32)
                    # val = -Cnorm/2 (hi+lo rows) + sum_j xT_j.T @ C_j
                    nc.tensor.matmul(
                        psum[:], lhsT=otile[:], rhs=btile[:], start=True, stop=False
                    )
                    for j in range(DCH):
                        nc.tensor.matmul(
                            psum[:],
                            lhsT=chunk(j, tl),
                            rhs=ctiles[j][:],
                            start=False,
                            stop=(j == DCH - 1),
                        )
                    if use_act_copy:
                        val = valp.tile([P, K], F32)
                        nc.scalar.copy(out=val[:], in_=psum[:])
                        src = val
                    else:
                        src = psum
                    mx = mxp.tile([P, 8], F32)
                    nc.vector.max(out=mx[:], in_=src[:])
                    nc.vector.max_index(
                        out=idxbuf[:, t, :], in_max=mx[:], in_values=src[:]
                    )
                    # stream tokens out every OSL tiles
                    if (t + 1) % OSL == 0:
                        s = t + 1 - OSL
                        nc.vector.tensor_copy(
                            out=tokbuf[:, s : t + 1], in_=idxbuf[:, s : t + 1, 0]
                        )
                        nc.sync.dma_start(
                            out=out[:, s : t + 1], in_=tokbuf[:, s : t + 1]
                        )

    _hoist_excess_waits(nc)
    return nc


def _hoist_excess_waits(nc: bass.Bass, max_waits: int = 1):
    """Hoist excess sync waits onto no-op drains inserted just before.

    Walrus's codegen caps embedded sync waits per instruction (1 for
    DIRECT2D DMAs and CTRL ops), but Tile can attach several (slot-reuse
    WAR + lane WAW, or the kernel-tail drain waiting on every proc).
    A same-engine drain immediately before the instruction blocks the
    sequencer at the same program point, so semantics are unchanged.
    """
    n = 0
    for f in nc.m.functions:
        for blk in f.blocks:
            insts = blk.instructions
            i = 0
            while i < len(insts):
                inst = insts[i]
                si = inst.sync_info
                if si and si.on_wait and len(si.on_wait) > max_waits:
                    waits = list(si.on_wait)
                    si.on_wait = waits[-max_waits:]
                    inst.sync_info = si
                    pre = []
                    for j in range(0, len(waits) - max_waits, max_waits):
                        nd = mybir.InstDrain(name=f"I-wsplit{n}", ins=[], outs=[])
                        n += 1
                        nd.engine = inst.engine
                        nsi = type(si)(
                            on_wait=waits[j : j + max_waits], on_update=[]
                        )
                        nd.sync_info = nsi
                        try:
                            nc.register_instruction(nd, overwrite=True)
                        except Exception:
                            pass
                        pre.append(nd)
                    for k, nd in enumerate(pre):
                        insts.insert(i + k, nd)
                    i += len(pre)
                i += 1


def make_in_maps(x: np.ndarray, C: np.ndarray, Cnorm: np.ndarray):
    x16 = x.astype(np.float16)
    C16 = C.astype(np.float16).reshape(DCH, P, K)

    bz = (-0.5 * Cnorm.reshape(K)).astype(np.float32)
    bh = bz.astype(np.float16)
    bl = (bz - bh.astype(np.float32)).astype(np.float16)

    cons = np.zeros((P, DCH * K + K + P), np.float16)
    cons[:, : DCH * K] = C16.transpose(1, 0, 2).reshape(P, DCH * K)
    cons[0, DCH * K : DCH * K + K] = bh
    cons[1, DCH * K : DCH * K + K] = bl
    cons[0:2, DCH * K + K :] = 1.0

    in_maps = []
    for c in range(NCORES):
        xs = x16[c * ROWS : (c + 1) * ROWS]
        # row r = p*128 + g*TPG + tl ; col = j*128 + pd
        xr = xs.reshape(P, GROUPS, TPG, DCH, P)          # [p, g, tl, j, pd]
        xgc = np.ascontiguousarray(xr.transpose(1, 4, 3, 2, 0))  # [g, pd, j, tl, p]
        in_maps.append(
            {
                "xg": xgc.reshape(GROUPS, P, DCH * TPG * P),
                "cons": cons,
            }
        )
    return in_maps


_NC_CACHE = {}


def kernel(x, C, Cnorm, b, t):
    global LAST_RESULT
    x = np.asarray(x)
    C = np.asarray(C)
    Cnorm = np.asarray(Cnorm)

    use_act_copy = bool(int(os.environ.get("KM_ACT_COPY", "0")))
    key = use_act_copy
    if key not in _NC_CACHE:
        _NC_CACHE[key] = build_nc(use_act_copy)
    nc = _NC_CACHE[key]

    in_maps = make_in_maps(x, C, Cnorm)
    trace = bool(int(os.environ.get("KM_TRACE", "0")))
    if trace:
        _ensure_ntff_hook()
    res = run_bass_kernel_spmd(
        nc, in_maps, core_ids=list(range(NCORES)), trace=trace
    )
    LAST_RESULT = res

    shards = [res.results[c]["out"].reshape(-1) for c in range(NCORES)]
    tokens = np.concatenate(shards).astype(np.int32)
    return tokens.reshape(int(b), int(t))
